# revision 1
# baseline (speedup 1.0000x reference)
"""Trainium2 Bass kernel for nn_Cycle_Consistency_Loss (soft-DTW-style
cycle loss). Self-contained: host-side packing + SPMD Bass program on 8
NeuronCores + host reduction.

Math (per pair (a,b), both directions; x = seq[q], y = seq[k], lens = src_len//4):
  alpha = softmax_j(-|x_i-y_j|^2) over valid j -> snn = alpha @ y
  beta  = softmax_k(-|snn_i-x_k|^2) over valid k
  u = E_beta[k], std = E_beta[(k-u)^2]
  li = (i-u)^2/std + 0.005*ln(std), summed over valid i; total / n_pairs.

Kernel decomposition: work items = 512-query blocks of each direction.
Per item, scores are computed transposed ([keys->partitions, queries->free])
via augmented matmuls so softmax denominators reduce over partitions on the
PE (no running max needed: pass-A scores <= 0; pass-B scores bounded).
Variance uses a two-round pass B (u first, then sum P2*(u-k)^2 elementwise)
to avoid catastrophic cancellation. Items are sorted by size and dealt
8-at-a-time into steps; loop bounds are compile-time per step.
"""
import sys
import numpy as np

sys.path.insert(0, "/opt/trn_rl_repo")

QB = 512          # query block = matmul free dim = one PSUM bank of fp32
KG = 256          # key group (2 chunks of 128 partitions)
NCORES = 8
PENALTY = 0.01
BIG = 1.0e30
STD_FLOOR = 1.0e-35


def _ceil(a, b):
    return -(-a // b)


class _Item:
    __slots__ = ("qi", "ki", "Lq", "Lk", "qb", "ga", "gb", "dummy")

    def __init__(self, qi, ki, Lq, Lk, qb):
        self.qi, self.ki, self.Lq, self.Lk, self.qb = qi, ki, Lq, Lk, qb
        self.ga = _ceil(Lk, KG)
        self.gb = _ceil(Lq, KG)
        self.dummy = False


class _Dummy:
    qi = ki = Lq = Lk = qb = 0
    ga = gb = 0
    dummy = True


def pack(seq, src_len, combinations):
    """Build the step plan and per-core input arrays.

    Per-core inputs (all fp32):
      kA  [34, CA]   pass-A key operand rows [yT; y2; 1] (masked keys y2=BIG)
      vAr [128, CA//128*33]  pass-A values, pre-swizzled so the on-chip
                     [128, 2GA, 33] tile loads with contiguous per-partition
                     rows: vAr[p, g*33+d] = vA[g*128+p, d], vA = [y | 1]
      qA  [34, QB*NS] pass-A query operand rows [2xT; -1; -x2]
      kB  [33, CB]   pass-B key operand rows [2xT; x2] (masked keys x2=BIG)
      kvo [128, 66]  col 2j = global key index of chunk j (j<32), col 2j+1 = 1;
                     col 64 = ones (sum-weights lhsT), col 65 = 0
      qidx/qmask [128, 4*NS] absolute query index / valid mask per B-slot
    """
    seq = np.asarray(seq, np.float32)
    lens = (np.asarray(src_len).astype(np.int64) // 4).astype(np.int64)
    comb = np.asarray(combinations).astype(np.int64)

    items = []
    for a, b in comb:
        for qi, ki in ((a, b), (b, a)):
            Lq, Lk = int(lens[qi]), int(lens[ki])
            if Lq <= 0 or Lk <= 0:
                continue
            for qb in range(_ceil(Lq, QB)):
                items.append(_Item(int(qi), int(ki), Lq, Lk, qb))
    items.sort(key=lambda it: -(it.ga + it.gb))
    NS = max(1, _ceil(len(items), NCORES))
    while len(items) < NS * NCORES:
        items.append(_Dummy())

    GA = [max(max(items[s * NCORES + c].ga for c in range(NCORES)), 1)
          for s in range(NS)]
    GB = [max(max(items[s * NCORES + c].gb for c in range(NCORES)), 1)
          for s in range(NS)]
    CA = sum(GA) * KG
    CB = sum(GB) * KG

    sq2 = np.einsum("btd,btd->bt", seq, seq).astype(np.float32)

    kvo = np.zeros((128, 66), np.float32)
    for j in range(32):
        kvo[:, 2 * j] = (j * 128 + np.arange(128)).astype(np.float32)
        kvo[:, 2 * j + 1] = 1.0
    kvo[:, 64] = 1.0

    cores = []
    for c in range(NCORES):
        kA = np.zeros((34, CA), np.float32)
        vA = np.zeros((CA, 33), np.float32)
        qA = np.zeros((34, QB * NS), np.float32)
        kB = np.zeros((33, CB), np.float32)
        qidx = np.zeros((128, 4 * NS), np.float32)
        qmask = np.zeros((128, 4 * NS), np.float32)
        offa = 0
        offb = 0
        its = []
        for s in range(NS):
            it = items[s * NCORES + c]
            its.append(it)
            na = GA[s] * KG
            nb = GB[s] * KG
            ka = kA[:, offa:offa + na]
            va = vA[offa:offa + na]
            kb = kB[:, offb:offb + nb]
            qa = qA[:, s * QB:(s + 1) * QB]
            if it.dummy:
                ka[33, :] = 1.0
                va[:, 32] = 1.0
            else:
                y = seq[it.ki]
                x = seq[it.qi]
                Lk, Lq = it.Lk, it.Lq
                nk = min(Lk, na)
                ka[0:32, :nk] = y[:nk].T
                ka[32, :nk] = sq2[it.ki, :nk]
                ka[33, :nk] = 1.0
                ka[32, nk:] = BIG
                ka[33, nk:] = 1.0
                va[:nk, 0:32] = y[:nk]
                va[:nk, 32] = 1.0
                q0 = it.qb * QB
                nq = min(Lq - q0, QB)
                qa[0:32, :nq] = 2.0 * x[q0:q0 + nq].T
                qa[32, :nq] = -1.0
                qa[33, :nq] = -sq2[it.qi, q0:q0 + nq]
                nkb = min(Lq, nb)
                kb[0:32, :nkb] = 2.0 * x[:nkb].T
                kb[32, :nkb] = sq2[it.qi, :nkb]
                kb[32, nkb:] = BIG
                for c4 in range(4):
                    ii = q0 + c4 * 128 + np.arange(128)
                    qidx[:, s * 4 + c4] = ii.astype(np.float32)
                    qmask[:, s * 4 + c4] = (ii < Lq).astype(np.float32)
            offa += na
            offb += nb
        vAr = np.ascontiguousarray(
            vA.reshape(CA // 128, 128, 33).transpose(1, 0, 2).reshape(128, -1))
        cores.append(dict(kA=kA, vAr=vAr, qA=qA, kB=kB, kvo=kvo,
                          qidx=qidx, qmask=qmask, items=its))
    plan = dict(NS=NS, GA=GA, GB=GB, CA=CA, CB=CB)
    return plan, cores


def build_program(plan):
    """Build the SPMD Bass program for the given step plan."""
    import concourse.bass as bass
    import concourse.bacc as bacc
    import concourse.mybir as mybir
    import concourse.tile as tile

    F32 = mybir.dt.float32
    AFT = mybir.ActivationFunctionType
    NS, GA, GB = plan["NS"], plan["GA"], plan["GB"]
    CA, CB = plan["CA"], plan["CB"]
    GBmax = max(GB)
    GAmax = max(GA)

    nc = bacc.Bacc("TRN2", target_bir_lowering=False, debug=False,
                   num_devices=NCORES)
    kA_d = nc.dram_tensor("kA", [34, CA], F32, kind="ExternalInput")
    vAr_d = nc.dram_tensor("vAr", [128, (CA // 128) * 33], F32,
                           kind="ExternalInput")
    qA_d = nc.dram_tensor("qA", [34, QB * NS], F32, kind="ExternalInput")
    kB_d = nc.dram_tensor("kB", [33, CB], F32, kind="ExternalInput")
    kvo_d = nc.dram_tensor("kvo", [128, 66], F32, kind="ExternalInput")
    qidx_d = nc.dram_tensor("qidx", [128, 4 * NS], F32, kind="ExternalInput")
    qmask_d = nc.dram_tensor("qmask", [128, 4 * NS], F32, kind="ExternalInput")
    out_d = nc.dram_tensor("out", [1, 1], F32, kind="ExternalOutput")

    with tile.TileContext(nc) as tc:
        with (
            tc.tile_pool(name="keys", bufs=2) as keys_pool,
            tc.tile_pool(name="vals", bufs=2) as vals_pool,
            tc.tile_pool(name="qrys", bufs=2) as qrys_pool,
            tc.tile_pool(name="pa", bufs=2) as pa_pool,
            tc.tile_pool(name="cache", bufs=1) as cache_pool,
            tc.tile_pool(name="epi", bufs=1) as epi_pool,
            tc.tile_pool(name="b2", bufs=2) as b2_pool,
            tc.tile_pool(name="fin", bufs=1) as fin_pool,
            tc.tile_pool(name="sc_ps", bufs=2, space="PSUM") as sc_psum,
            tc.tile_pool(name="na_ps", bufs=1, space="PSUM") as na_psum,
            tc.tile_pool(name="t_ps", bufs=1, space="PSUM") as t_psum,
            tc.tile_pool(name="sd_ps", bufs=1, space="PSUM") as sd_psum,
        ):
            kvo = fin_pool.tile([128, 66], F32)
            nc.sync.dma_start(kvo[:], kvo_d[:])
            qidx = fin_pool.tile([128, 4 * NS], F32)
            nc.sync.dma_start(qidx[:], qidx_d[:])
            qmask = fin_pool.tile([128, 4 * NS], F32)
            nc.sync.dma_start(qmask[:], qmask_d[:])
            stats_u = fin_pool.tile([128, 4 * NS], F32)
            stats_s = fin_pool.tile([128, 4 * NS], F32)

            offa = 0
            offb = 0
            for s in range(NS):
                ga, gb = GA[s], GB[s]
                na, nb = ga * KG, gb * KG
                # ---- load this step's operands
                kA_t = keys_pool.tile([34, GAmax * KG], F32, tag="kA")
                nc.sync.dma_start(kA_t[:, :na], kA_d[:, offa:offa + na])
                vA_t = vals_pool.tile([128, GAmax * 2 * 33], F32, tag="vA")
                nc.sync.dma_start(
                    vA_t[:, :ga * 66],
                    vAr_d[:, (offa // 128) * 33:((offa + na) // 128) * 33])
                qA_t = qrys_pool.tile([34, QB], F32, tag="qA")
                nc.sync.dma_start(qA_t[:], qA_d[:, s * QB:(s + 1) * QB])
                kB_t = keys_pool.tile([33, GBmax * KG], F32, tag="kB")
                nc.sync.dma_start(kB_t[:, :nb], kB_d[:, offb:offb + nb])

                # ---- pass A: numA[0:32] = snn.T * Z, numA[32] = Z
                numA = na_psum.tile([33, QB], F32)
                for g in range(ga):
                    sc = sc_psum.tile([128, 2 * QB], F32, tag="sc")
                    P = pa_pool.tile([128, 2 * QB], F32, tag="pa")
                    for h in range(2):
                        ch = 2 * g + h
                        nc.tensor.matmul(
                            sc[:, h * QB:(h + 1) * QB],
                            kA_t[:, ch * 128:(ch + 1) * 128], qA_t[:],
                            start=True, stop=True)
                    nc.scalar.activation(P[:], sc[:], AFT.Exp)
                    for h in range(2):
                        ch = 2 * g + h
                        nc.tensor.matmul(
                            numA[:],
                            vA_t[:, ch * 33:(ch + 1) * 33],
                            P[:, h * QB:(h + 1) * QB],
                            start=(g == 0 and h == 0),
                            stop=(g == ga - 1 and h == 1))

                # ---- epilogue A: R2 = [snn.T; -1]
                nsb = epi_pool.tile([33, QB], F32, tag="nsb")
                nc.vector.tensor_copy(nsb[:], numA[:])
                zrow = epi_pool.tile([1, QB], F32, tag="zrow")
                nc.sync.dma_start(zrow[:], nsb[32:33, :])
                rz0 = epi_pool.tile([1, QB], F32, tag="rz0")
                nc.vector.reciprocal(rz0[:], zrow[:])
                rb = epi_pool.tile([32, QB], F32, tag="rb")
                nc.gpsimd.partition_broadcast(rb[:], rz0[:])
                R2 = epi_pool.tile([33, QB], F32, tag="R2")
                nc.gpsimd.memset(R2[32:33, :], -1.0)
                nc.vector.tensor_mul(R2[0:32, :], nsb[0:32, :], rb[:])

                # ---- pass B1: P2 cached; T = [r0; Z2]
                cache = cache_pool.tile([128, GBmax * 2 * QB], F32, tag="p2c")
                T = t_psum.tile([2, QB], F32, tag="T")
                for g in range(gb):
                    sc = sc_psum.tile([128, 2 * QB], F32, tag="sc")
                    for h in range(2):
                        ch = 2 * g + h
                        nc.tensor.matmul(
                            sc[:, h * QB:(h + 1) * QB],
                            kB_t[:, ch * 128:(ch + 1) * 128], R2[:],
                            start=True, stop=True)
                    nc.scalar.activation(
                        cache[:, g * 2 * QB:(g + 1) * 2 * QB], sc[:], AFT.Exp)
                    for h in range(2):
                        ch = 2 * g + h
                        nc.tensor.matmul(
                            T[:],
                            kvo[:, 2 * ch:2 * ch + 2],
                            cache[:, (2 * g + h) * QB:(2 * g + h + 1) * QB],
                            start=(g == 0 and h == 0),
                            stop=(g == gb - 1 and h == 1))

                # ---- mid: u = r0 / Z2, broadcast
                tt = epi_pool.tile([2, QB], F32, tag="tt")
                nc.vector.tensor_copy(tt[:], T[:])
                z2row = epi_pool.tile([1, QB], F32, tag="z2row")
                nc.sync.dma_start(z2row[:], tt[1:2, :])
                rz2 = epi_pool.tile([1, QB], F32, tag="rz2")
                nc.vector.reciprocal(rz2[:], z2row[:])
                u0 = epi_pool.tile([1, QB], F32, tag="u0")
                nc.vector.tensor_mul(u0[:], tt[0:1, :], rz2[:])
                ub = epi_pool.tile([128, QB], F32, tag="ub")
                nc.gpsimd.partition_broadcast(ub[:], u0[:])

                # ---- pass B2: stdsum = sum_k P2 * (u-k)^2
                stdsum = sd_psum.tile([1, QB], F32, tag="sd")
                for g in range(gb):
                    for h in range(2):
                        ch = 2 * g + h
                        d = b2_pool.tile([128, QB], F32, tag="d")
                        nc.vector.tensor_scalar_sub(
                            d[:], ub[:], kvo[:, 2 * ch:2 * ch + 1])
                        sq = b2_pool.tile([128, QB], F32, tag="sq")
                        nc.vector.tensor_mul(sq[:], d[:], d[:])
                        w = b2_pool.tile([128, QB], F32, tag="w")
                        nc.gpsimd.tensor_mul(
                            w[:], sq[:],
                            cache[:, (2 * g + h) * QB:(2 * g + h + 1) * QB])
                        nc.tensor.matmul(
                            stdsum[:], kvo[:, 64:65], w[:],
                            start=(g == 0 and h == 0),
                            stop=(g == gb - 1 and h == 1))

                # ---- epilogue B: write u, std into stats via transpose-DMA
                sstd = epi_pool.tile([1, QB], F32, tag="sstd")
                nc.vector.tensor_mul(sstd[:], stdsum[:], rz2[:])
                for c4 in range(4):
                    nc.sync.dma_start(
                        stats_u[:, s * 4 + c4:s * 4 + c4 + 1],
                        u0[0:1, c4 * 128:(c4 + 1) * 128])
                    nc.sync.dma_start(
                        stats_s[:, s * 4 + c4:s * 4 + c4 + 1],
                        sstd[0:1, c4 * 128:(c4 + 1) * 128])
                offa += na
                offb += nb

            # ---- final: li = (i-u)^2/std + 0.005*ln(std), masked sum
            W = 4 * NS
            stdc = fin_pool.tile([128, W], F32)
            nc.vector.tensor_scalar_max(stdc[:], stats_s[:], STD_FLOOR)
            rstd = fin_pool.tile([128, W], F32)
            nc.vector.reciprocal(rstd[:], stdc[:])
            delta = fin_pool.tile([128, W], F32)
            nc.vector.tensor_sub(delta[:], qidx[:], stats_u[:])
            d2 = fin_pool.tile([128, W], F32)
            nc.vector.tensor_mul(d2[:], delta[:], delta[:])
            t1 = fin_pool.tile([128, W], F32)
            nc.vector.tensor_mul(t1[:], d2[:], rstd[:])
            lg = fin_pool.tile([128, W], F32)
            nc.scalar.activation(lg[:], stdc[:], AFT.Ln)
            lgs = fin_pool.tile([128, W], F32)
            nc.vector.tensor_scalar_mul(lgs[:], lg[:], 0.5 * PENALTY)
            li = fin_pool.tile([128, W], F32)
            nc.vector.tensor_add(li[:], t1[:], lgs[:])
            lim = fin_pool.tile([128, W], F32)
            nc.vector.tensor_mul(lim[:], li[:], qmask[:])
            rowsum = fin_pool.tile([128, 1], F32)
            nc.vector.reduce_sum(rowsum[:], lim[:],
                                 axis=mybir.AxisListType.X)
            tot = t_psum.tile([1, 1], F32, tag="tot")
            nc.tensor.matmul(tot[:], rowsum[:], kvo[0:128, 64:65],
                             start=True, stop=True)
            osb = fin_pool.tile([1, 1], F32)
            nc.vector.tensor_copy(osb[:], tot[:])
            nc.sync.dma_start(out_d[:], osb[:])

    nc.compile()
    return nc


def kernel(seq, src_len, combinations):
    from concourse.bass_utils import run_bass_kernel_spmd

    plan, cores = pack(seq, src_len, combinations)
    nc = build_program(plan)
    in_maps = [
        {k: ci[k] for k in
         ("kA", "vAr", "qA", "kB", "kvo", "qidx", "qmask")}
        for ci in cores
    ]
    res = run_bass_kernel_spmd(nc, in_maps, list(range(NCORES)))
    tot = np.float32(0.0)
    for c in range(NCORES):
        tot += np.float32(res.results[c]["out"][0, 0])
    n_pairs = np.asarray(combinations).shape[0]
    return np.float32(tot / np.float32(n_pairs))



# revision 24
# speedup vs baseline: 2.8942x; 2.8942x over previous
"""Trainium2 Bass kernel for nn_Cycle_Consistency_Loss (soft-DTW-style
cycle loss). Self-contained: host-side packing + SPMD Bass program on 8
NeuronCores + host reduction.

Math (per pair (a,b), both directions; x = seq[q], y = seq[k], lens = src_len//4):
  alpha = softmax_j(-|x_i-y_j|^2) over valid j -> snn = alpha @ y
  beta  = softmax_k(-|snn_i-x_k|^2) over valid k
  u = E_beta[k], std = E_beta[(k-u)^2]
  li = (i-u)^2/std + 0.005*ln(std), summed over valid i; total / n_pairs.

Kernel decomposition: work items = 512-query blocks of each direction.
Scores are computed transposed ([keys->partitions, queries->free]) via
augmented matmuls so softmax denominators reduce over partitions on the PE.

Precision plan (validated in numpy sim, rel err ~1.4e-3 vs fp32 ref):
  - score matmuls: fp16 operands (1-pass PE), squared-norm rows split
    hi/lo so their fp16 rounding stays ~1e-5 absolute.
  - softmax weights P/P2: bf16 (range needed: P2 up to e^88).
  - index moments: lhsT rows [1; 256*(k>>8); k&255] (bf16-exact ints).
  - variance pass: sq = Square(u - k) on ACT (fp32 internal, bf16 out,
    per-partition bias = -k), w = sq*P2 on DVE (bf16 2x), sum via PE.
  - 1/Z via exp(-ln(Z)) on ACT (DVE divide is 8 cyc/elem).
Step structure is software-pipelined: B2 of step s-1 is emitted between
pass A and pass B1 of step s so PE has independent work while the
per-item epilogue chains (Ln/Exp/broadcast) resolve.
"""
import sys
import numpy as np

sys.path.insert(0, "/opt/trn_rl_repo")

QB = 512          # query block = matmul free dim = one PSUM bank of fp32
KG = 256          # key group (2 chunks of 128 partitions)
NCORES = 8
PENALTY = 0.01
BIGM = 60000.0    # fp16-representable mask value
STD_FLOOR = 1.0e-35

INPUT_NAMES = ("kA", "vAr", "qA", "kB", "kmom", "kneg", "qidx", "qmask")


def _ceil(a, b):
    return -(-a // b)


class _Item:
    __slots__ = ("qi", "ki", "Lq", "Lk", "qb", "ga", "gb", "dummy")

    def __init__(self, qi, ki, Lq, Lk, qb):
        self.qi, self.ki, self.Lq, self.Lk, self.qb = qi, ki, Lq, Lk, qb
        self.ga = _ceil(Lk, KG)
        self.gb = _ceil(Lq, KG)
        self.dummy = False


class _Dummy:
    qi = ki = Lq = Lk = qb = 0
    ga = gb = 0
    dummy = True


def _pack_steps(items, ns):
    """Greedy 2-D packing: assign items to ns steps x 8 cores minimizing
    sum_s max_core(ga) + 1.6 * sum_s max_core(gb)."""
    CB = 1.6
    steps = [[] for _ in range(ns)]
    ga_max = [0] * ns
    gb_max = [0] * ns
    for it in sorted(items, key=lambda t: -(t.ga + CB * t.gb)):
        best, bcost = None, None
        for s in range(ns):
            if len(steps[s]) >= NCORES:
                continue
            cost = (max(ga_max[s], it.ga) - ga_max[s]
                    + CB * (max(gb_max[s], it.gb) - gb_max[s]))
            # tie-break: prefer fuller steps to leave empty slots together
            key = (cost, -len(steps[s]))
            if bcost is None or key < bcost:
                bcost, best = key, s
        steps[best].append(it)
        ga_max[best] = max(ga_max[best], it.ga)
        gb_max[best] = max(gb_max[best], it.gb)
    for s in range(ns):
        while len(steps[s]) < NCORES:
            steps[s].append(_Dummy())
    return steps


def _split_hi_lo16(v):
    hi = v.astype(np.float16).astype(np.float32)
    lo = (v - hi).astype(np.float16).astype(np.float32)
    return hi, lo


def pack(seq, src_len, combinations):
    """Build the step plan and per-core input arrays.

    Per-core inputs:
      kA   fp16 [36, CA]    pass-A keys [y; y2h; y2l; 1; 1] (masked y2h=BIGM)
      vAr  fp16 [128, CA//128*33]  pass-A values pre-swizzled:
                            vAr[p, c*33+d] = vA[c*128+p, d], vA = [y | 1]
      qA   fp16 [36, QB*NS] pass-A queries [2xT; -1; -1; -x2h; -x2l]
      kB   fp16 [34, CB]    pass-B keys [2xT; x2h; x2l] (masked x2h=BIGM)
      kmom bf16 [128, 32*3] moment lhsT per chunk: [1, 256*(k>>8), k&255]
      kneg fp32 [128, 33]   col ch = -(128*ch+p) (ACT Square bias); col 32 = 1.0
      qidx/qmask fp32 [128, 4*NS] absolute query index / valid mask
    """
    seq = np.asarray(seq, np.float32)
    lens = (np.asarray(src_len).astype(np.int64) // 4).astype(np.int64)
    comb = np.asarray(combinations).astype(np.int64)

    items = []
    for a, b in comb:
        for qi, ki in ((a, b), (b, a)):
            Lq, Lk = int(lens[qi]), int(lens[ki])
            if Lq <= 0 or Lk <= 0:
                continue
            for qb in range(_ceil(Lq, QB)):
                items.append(_Item(int(qi), int(ki), Lq, Lk, qb))
    NS = max(1, _ceil(len(items), NCORES))
    steps = _pack_steps(items, NS)

    GA = [max(max(it.ga for it in steps[s]), 1) for s in range(NS)]
    GB = [max(max(it.gb for it in steps[s]), 1) for s in range(NS)]
    CA = sum(GA) * KG
    CB = sum(GB) * KG

    sq2 = np.einsum("btd,btd->bt", seq, seq).astype(np.float32)

    # T-matmul lhsT: 65 cols/chunk so output rows land on partitions
    # 0/32/64 (compute-engine APs need 32-aligned partition starts)
    kk = np.arange(32 * 128, dtype=np.int64)
    kmom = np.zeros((128, 32 * 65), np.float32)
    kmom[:, 0::65] = 1.0
    kmom[:, 32::65] = (256 * (kk >> 8)).astype(np.float32).reshape(32, 128).T
    kmom[:, 64::65] = (kk & 255).astype(np.float32).reshape(32, 128).T
    kneg = np.zeros((128, 33), np.float32)
    kneg[:, :32] = -kk.astype(np.float32).reshape(32, 128).T
    kneg[:, 32] = 1.0

    cores = []
    for c in range(NCORES):
        kA = np.zeros((36, CA), np.float32)
        vA = np.zeros((CA, 33), np.float32)
        qA = np.zeros((36, QB * NS), np.float32)
        kB = np.zeros((34, CB), np.float32)
        qidx = np.zeros((128, 4 * NS), np.float32)
        qmask = np.zeros((128, 4 * NS), np.float32)
        offa = 0
        offb = 0
        its = []
        for s in range(NS):
            it = steps[s][c]
            its.append(it)
            na = GA[s] * KG
            nb = GB[s] * KG
            ka = kA[:, offa:offa + na]
            va = vA[offa:offa + na]
            kb = kB[:, offb:offb + nb]
            qa = qA[:, s * QB:(s + 1) * QB]
            if it.dummy:
                # scores 0 -> P = 1; Z = na via vA flag column
                va[:, 32] = 1.0
            else:
                y = seq[it.ki]
                x = seq[it.qi]
                Lk, Lq = it.Lk, it.Lq
                nk = min(Lk, na)
                ka[0:32, :nk] = y[:nk].T
                y2h, y2l = _split_hi_lo16(sq2[it.ki, :nk])
                ka[32, :nk] = y2h
                ka[33, :nk] = y2l
                ka[32, nk:] = BIGM
                ka[34, :nk] = 1.0
                ka[35, :nk] = 1.0
                va[:nk, 0:32] = y[:nk]
                va[:nk, 32] = 1.0
                q0 = it.qb * QB
                nq = min(Lq - q0, QB)
                qa[0:32, :nq] = 2.0 * x[q0:q0 + nq].T
                qa[32, :nq] = -1.0
                qa[33, :nq] = -1.0
                x2h, x2l = _split_hi_lo16(sq2[it.qi, q0:q0 + nq])
                qa[34, :nq] = -x2h
                qa[35, :nq] = -x2l
                nkb = min(Lq, nb)
                kb[0:32, :nkb] = 2.0 * x[:nkb].T
                xh, xl = _split_hi_lo16(sq2[it.qi, :nkb])
                kb[32, :nkb] = xh
                kb[33, :nkb] = xl
                kb[32, nkb:] = BIGM
                for c4 in range(4):
                    ii = q0 + c4 * 128 + np.arange(128)
                    qidx[:, s * 4 + c4] = ii.astype(np.float32)
                    qmask[:, s * 4 + c4] = (ii < Lq).astype(np.float32)
            offa += na
            offb += nb
        vAr = np.ascontiguousarray(
            vA.reshape(CA // 128, 128, 33).transpose(1, 0, 2).reshape(128, -1))

        import ml_dtypes
        cores.append(dict(
            kA=kA.astype(np.float16),
            vAr=vAr.astype(np.float16),
            qA=qA.astype(np.float16),
            kB=kB.astype(np.float16),
            kmom=kmom.astype(ml_dtypes.bfloat16),
            kneg=kneg,
            qidx=qidx, qmask=qmask, items=its))
    plan = dict(NS=NS, GA=GA, GB=GB, CA=CA, CB=CB)
    return plan, cores


def build_program(plan, debug=None):
    """Build the SPMD Bass program for the given step plan."""
    import concourse.bass as bass
    import concourse.bacc as bacc
    import concourse.mybir as mybir
    import concourse.tile as tile

    F32 = mybir.dt.float32
    F16 = mybir.dt.float16
    BF16 = mybir.dt.bfloat16
    AFT = mybir.ActivationFunctionType
    NS, GA, GB = plan["NS"], plan["GA"], plan["GB"]
    CA, CB = plan["CA"], plan["CB"]
    GBmax = max(GB)
    GAmax = max(GA)

    nc = bacc.Bacc("TRN2", target_bir_lowering=False, debug=False,
                   num_devices=NCORES)
    kA_d = nc.dram_tensor("kA", [36, CA], F16, kind="ExternalInput")
    vAr_d = nc.dram_tensor("vAr", [128, (CA // 128) * 33], F16,
                           kind="ExternalInput")
    qA_d = nc.dram_tensor("qA", [36, QB * NS], F16, kind="ExternalInput")
    kB_d = nc.dram_tensor("kB", [34, CB], F16, kind="ExternalInput")
    kmom_d = nc.dram_tensor("kmom", [128, 32 * 65], BF16, kind="ExternalInput")
    kneg_d = nc.dram_tensor("kneg", [128, 33], F32, kind="ExternalInput")
    qidx_d = nc.dram_tensor("qidx", [128, 4 * NS], F32, kind="ExternalInput")
    qmask_d = nc.dram_tensor("qmask", [128, 4 * NS], F32, kind="ExternalInput")
    out_d = nc.dram_tensor("out", [1, 1], F32, kind="ExternalOutput")
    if debug is not None:
        dbgA_d = nc.dram_tensor("dbgA", [33, QB], F32, kind="ExternalOutput")
        dbgR_d = nc.dram_tensor("dbgR", [34, QB], F32, kind="ExternalOutput")
        dbgT_d = nc.dram_tensor("dbgT", [65, QB], F32, kind="ExternalOutput")
        dbgU_d = nc.dram_tensor("dbgU", [2, QB], F32, kind="ExternalOutput")
        dbgSu_d = nc.dram_tensor("dbgSu", [128, 4 * plan["NS"]], F32,
                                 kind="ExternalOutput")
        dbgSs_d = nc.dram_tensor("dbgSs", [128, 4 * plan["NS"]], F32,
                                 kind="ExternalOutput")

    with tile.TileContext(nc) as tc:
        with (
            tc.tile_pool(name="keys", bufs=2) as keys_pool,
            tc.tile_pool(name="vals", bufs=2) as vals_pool,
            tc.tile_pool(name="qrys", bufs=2) as qrys_pool,
            tc.tile_pool(name="pa", bufs=3) as pa_pool,
            tc.tile_pool(name="cache", bufs=2) as cache_pool,
            tc.tile_pool(name="epi", bufs=2) as epi_pool,
            tc.tile_pool(name="r2", bufs=2) as r2_pool,
            tc.tile_pool(name="b2", bufs=3) as b2_pool,
            tc.tile_pool(name="fin", bufs=1) as fin_pool,
            tc.tile_pool(name="sc_ps", bufs=2, space="PSUM") as sc_psum,
            tc.tile_pool(name="na_ps", bufs=1, space="PSUM") as na_psum,
            tc.tile_pool(name="t_ps", bufs=1, space="PSUM") as t_psum,
            tc.tile_pool(name="sd_ps", bufs=1, space="PSUM") as sd_psum,
        ):
            kmom = fin_pool.tile([128, 32 * 65], BF16)
            nc.sync.dma_start(kmom[:], kmom_d[:])
            kneg = fin_pool.tile([128, 33], F32)
            nc.sync.dma_start(kneg[:], kneg_d[:])
            qidx = fin_pool.tile([128, 4 * NS], F32)
            nc.sync.dma_start(qidx[:], qidx_d[:])
            qmask = fin_pool.tile([128, 4 * NS], F32)
            nc.sync.dma_start(qmask[:], qmask_d[:])
            stats_u = fin_pool.tile([128, 4 * NS], F32)
            stats_s = fin_pool.tile([128, 4 * NS], F32)
            onesbf = fin_pool.tile([128, 1], BF16)
            nc.gpsimd.memset(onesbf[:], 1.0)

            # per-step state carried from B1(s) into B2(s) emitted next iter
            carry = None  # (gb, cache, u0, rz2, sstep)

            offa = 0
            offb = 0
            for s in range(NS + 1):
                if s < NS:
                    ga, gb = GA[s], GB[s]
                    na, nb = ga * KG, gb * KG
                    # ---- load this step's operands
                    kA_t = keys_pool.tile([36, GAmax * KG], F16, tag="kA")
                    nc.sync.dma_start(kA_t[:, :na], kA_d[:, offa:offa + na])
                    vA_t = vals_pool.tile([128, GAmax * 66], F16, tag="vA")
                    nc.sync.dma_start(
                        vA_t[:, :ga * 66],
                        vAr_d[:, (offa // 128) * 33:((offa + na) // 128) * 33])
                    qA_t = qrys_pool.tile([36, QB], F16, tag="qA")
                    nc.sync.dma_start(qA_t[:], qA_d[:, s * QB:(s + 1) * QB])
                    kB_t = keys_pool.tile([34, GBmax * KG], F16, tag="kB")
                    nc.sync.dma_start(kB_t[:, :nb], kB_d[:, offb:offb + nb])

                    # ---- pass A (skew-2: snn MMs for group g-2 follow
                    # score MMs for group g so PE never waits on exp)
                    numA = na_psum.tile([33, QB], F32)
                    Ps = [None, None, None]

                    def snn_mms(g):
                        P = Ps[g % 3]
                        for h in range(2):
                            nc.tensor.matmul(
                                numA[:],
                                vA_t[:, (2 * g + h) * 33:(2 * g + h + 1) * 33],
                                P[:, h * QB:(h + 1) * QB],
                                start=(g == 0 and h == 0),
                                stop=(g == ga - 1 and h == 1))

                    for g in range(ga):
                        sc = sc_psum.tile([128, 2 * QB], F32, tag="sc")
                        for h in range(2):
                            ch = 2 * g + h
                            nc.tensor.matmul(
                                sc[:, h * QB:(h + 1) * QB],
                                kA_t[:, ch * 128:(ch + 1) * 128], qA_t[:],
                                start=True, stop=True)
                        if g >= 2:
                            snn_mms(g - 2)
                        P = pa_pool.tile([128, 2 * QB], BF16, tag="pa")
                        Ps[g % 3] = P
                        nc.scalar.activation(P[:], sc[:], AFT.Exp)
                    for g in range(max(0, ga - 2), ga):
                        snn_mms(g)

                    # ---- epilogue A: R2 = [snn/Z; -1; -1] fp16
                    lnz = epi_pool.tile([1, QB], F32, tag="lnz")
                    nc.scalar.activation(lnz[:], numA[32:33, :], AFT.Ln)
                    rz = epi_pool.tile([1, QB], F32, tag="rz")
                    nc.scalar.activation(rz[:], lnz[:], AFT.Exp, scale=-1.0)
                    rb = epi_pool.tile([32, QB], F32, tag="rb")
                    nc.gpsimd.partition_broadcast(rb[:], rz[:])
                    R2 = r2_pool.tile([34, QB], F16, tag="R2")
                    nc.gpsimd.memset(R2[32:34, :], -1.0)
                    nc.vector.tensor_mul(R2[0:32, :], numA[0:32, :], rb[:])
                    if debug is not None and s == debug:
                        dA = epi_pool.tile([33, QB], F32, tag="dA")
                        nc.vector.tensor_copy(dA[:], numA[:])
                        nc.sync.dma_start(dbgA_d[:], dA[:])
                        dR = epi_pool.tile([34, QB], F32, tag="dR")
                        nc.vector.tensor_copy(dR[:], R2[:])
                        nc.sync.dma_start(dbgR_d[:], dR[:])

                # ---- pass B2 of the PREVIOUS step (pipelined)
                if carry is not None:
                    pgb, pcache, pu0, prz2, ps = carry
                    ubp = b2_pool.tile([128, QB], F32, tag="ub")
                    nc.gpsimd.partition_broadcast(ubp[:], pu0[:])
                    sd = sd_psum.tile([1, QB], F32, tag="sd")
                    for ch in range(2 * pgb):
                        sq = b2_pool.tile([128, QB], BF16, tag="sq")
                        if ch % 2 == 0:
                            nc.scalar.activation(
                                sq[:], ubp[:], AFT.Square,
                                bias=kneg[:, ch:ch + 1])
                        else:
                            dt = b2_pool.tile([128, QB], F16, tag="dt")
                            nc.vector.tensor_scalar_add(
                                dt[:], ubp[:], kneg[:, ch:ch + 1])
                            nc.vector.tensor_mul(sq[:], dt[:], dt[:])
                        w = b2_pool.tile([128, QB], BF16, tag="w")
                        nc.vector.tensor_mul(
                            w[:], sq[:],
                            pcache[:, ch * QB:(ch + 1) * QB])
                        nc.tensor.matmul(
                            sd[:], onesbf[:], w[:],
                            start=(ch == 0), stop=(ch == 2 * pgb - 1))
                    sstd = epi_pool.tile([1, QB], F32, tag="sstd")
                    nc.vector.tensor_mul(sstd[:], sd[:], prz2[:])
                    for c4 in range(4):
                        nc.sync.dma_start(
                            stats_u[:, ps * 4 + c4:ps * 4 + c4 + 1],
                            pu0[0:1, c4 * 128:(c4 + 1) * 128])
                        nc.sync.dma_start(
                            stats_s[:, ps * 4 + c4:ps * 4 + c4 + 1],
                            sstd[0:1, c4 * 128:(c4 + 1) * 128])
                    carry = None

                if s >= NS:
                    break

                # ---- pass B1: cache P2 (bf16), T = [Z2; 256a; b] (skew-2)
                cache = cache_pool.tile([128, GBmax * 2 * QB], BF16, tag="p2c")
                T = t_psum.tile([65, QB], F32, tag="T")

                def t_mms(g):
                    for h in range(2):
                        ch = 2 * g + h
                        nc.tensor.matmul(
                            T[:],
                            kmom[:, 65 * ch:65 * ch + 65],
                            cache[:, ch * QB:(ch + 1) * QB],
                            start=(g == 0 and h == 0),
                            stop=(g == gb - 1 and h == 1))

                for g in range(gb):
                    sc = sc_psum.tile([128, 2 * QB], F32, tag="sc")
                    for h in range(2):
                        ch = 2 * g + h
                        nc.tensor.matmul(
                            sc[:, h * QB:(h + 1) * QB],
                            kB_t[:, ch * 128:(ch + 1) * 128], R2[:],
                            start=True, stop=True)
                    if g >= 2:
                        t_mms(g - 2)
                    nc.scalar.activation(
                        cache[:, g * 2 * QB:(g + 1) * 2 * QB], sc[:], AFT.Exp)
                for g in range(max(0, gb - 2), gb):
                    t_mms(g)

                # ---- epilogue B1: u = (T1 + T2) / Z2.  1/Z2 via DVE
                # reciprocal on a transposed [128,4] layout (Z2 reaches
                # ~1e26; ACT Ln breaks above ~1e19, and a [1,512] DVE
                # reciprocal costs 8 cyc/elem = 4.3us)
                z2r = epi_pool.tile([1, QB], F32, tag="z2r")
                nc.vector.tensor_copy(z2r[:], T[0:1, :])
                z2t = epi_pool.tile([128, 4], F32, tag="z2t")
                for c4 in range(4):
                    nc.sync.dma_start(z2t[:, c4:c4 + 1],
                                      z2r[0:1, c4 * 128:(c4 + 1) * 128])
                rz2t = epi_pool.tile([128, 4], F32, tag="rz2t")
                nc.vector.reciprocal(rz2t[:], z2t[:])
                rz2 = epi_pool.tile([1, QB], F32, tag="rz2")
                for c4 in range(4):
                    nc.sync.dma_start(rz2[0:1, c4 * 128:(c4 + 1) * 128],
                                      rz2t[:, c4:c4 + 1])
                ta0 = epi_pool.tile([1, QB], F32, tag="ta0")
                nc.vector.tensor_copy(ta0[:], T[32:33, :])
                r0 = epi_pool.tile([1, QB], F32, tag="r0")
                nc.vector.tensor_add(r0[:], ta0[:], T[64:65, :])
                u0 = epi_pool.tile([1, QB], F32, tag="u0")
                nc.vector.tensor_mul(u0[:], r0[:], rz2[:])

                if debug is not None and s == debug:
                    dT = epi_pool.tile([65, QB], F32, tag="dT")
                    nc.vector.tensor_copy(dT[:], T[:])
                    nc.sync.dma_start(dbgT_d[:], dT[:])
                    nc.sync.dma_start(dbgU_d[0:1, :], u0[:])
                    nc.sync.dma_start(dbgU_d[1:2, :], rz2[:])

                carry = (gb, cache, u0, rz2, s)
                offa += na
                offb += nb

            # ---- final: li = (i-u)^2/std + 0.005*ln(std), masked sum
            W = 4 * NS
            stdc = fin_pool.tile([128, W], F32)
            nc.vector.tensor_scalar_max(stdc[:], stats_s[:], STD_FLOOR)
            rstd = fin_pool.tile([128, W], F32)
            nc.vector.reciprocal(rstd[:], stdc[:])
            delta = fin_pool.tile([128, W], F32)
            nc.vector.tensor_sub(delta[:], qidx[:], stats_u[:])
            d2 = fin_pool.tile([128, W], F32)
            nc.vector.tensor_mul(d2[:], delta[:], delta[:])
            t1 = fin_pool.tile([128, W], F32)
            nc.vector.tensor_mul(t1[:], d2[:], rstd[:])
            lg = fin_pool.tile([128, W], F32)
            nc.scalar.activation(lg[:], stdc[:], AFT.Ln)
            lgs = fin_pool.tile([128, W], F32)
            nc.vector.tensor_scalar_mul(lgs[:], lg[:], 0.5 * PENALTY)
            li = fin_pool.tile([128, W], F32)
            nc.vector.tensor_add(li[:], t1[:], lgs[:])
            lim = fin_pool.tile([128, W], F32)
            nc.vector.tensor_mul(lim[:], li[:], qmask[:])
            if debug is not None:
                nc.sync.dma_start(dbgSu_d[:], stats_u[:])
                nc.sync.dma_start(dbgSs_d[:], stats_s[:])
            rowsum = fin_pool.tile([128, 1], F32)
            nc.vector.reduce_sum(rowsum[:], lim[:],
                                 axis=mybir.AxisListType.X)
            tot = t_psum.tile([1, 1], F32, tag="tot")
            nc.tensor.matmul(tot[:], rowsum[:], kneg[0:128, 32:33],
                             start=True, stop=True)
            osb = fin_pool.tile([1, 1], F32)
            nc.vector.tensor_copy(osb[:], tot[:])
            nc.sync.dma_start(out_d[:], osb[:])

    nc.compile()
    return nc


def kernel(seq, src_len, combinations):
    from concourse.bass_utils import run_bass_kernel_spmd

    plan, cores = pack(seq, src_len, combinations)
    nc = build_program(plan)
    in_maps = [{k: ci[k] for k in INPUT_NAMES} for ci in cores]
    res = run_bass_kernel_spmd(nc, in_maps, list(range(NCORES)))
    tot = np.float32(0.0)
    for c in range(NCORES):
        tot += np.float32(res.results[c]["out"][0, 0])
    n_pairs = np.asarray(combinations).shape[0]
    return np.float32(tot / np.float32(n_pairs))


# revision 25
# speedup vs baseline: 3.2229x; 1.1136x over previous
"""Trainium2 Bass kernel for nn_Cycle_Consistency_Loss (soft-DTW-style
cycle loss). Self-contained: host-side packing + SPMD Bass program on 8
NeuronCores + host reduction.

Math (per pair (a,b), both directions; x = seq[q], y = seq[k], lens = src_len//4):
  alpha = softmax_j(-|x_i-y_j|^2) over valid j -> snn = alpha @ y
  beta  = softmax_k(-|snn_i-x_k|^2) over valid k
  u = E_beta[k], std = E_beta[(k-u)^2]
  li = (i-u)^2/std + 0.005*ln(std), summed over valid i; total / n_pairs.

Kernel decomposition: work items = 512-query blocks of each direction.
Scores are computed transposed ([keys->partitions, queries->free]) via
augmented matmuls so softmax denominators reduce over partitions on the PE.

Precision plan (validated in numpy sim, rel err ~1.7e-3 vs fp32 ref):
  - score matmuls: fp16 operands (1-pass PE), squared-norm rows split
    hi/lo so their fp16 rounding stays ~1e-5 absolute.
  - softmax weights P/P2: bf16 out of ACT Exp (range: P2 up to ~e^64).
  - pass-B stats via per-chunk moment matmuls: lhsT cols per chunk =
    [1, -2p, 128*(p^2>>7), p^2&127] (bf16-exact), accumulated into one
    [128,512] PSUM tile with chunk ch's rows at partitions ch/32+ch/
    64+ch/96+ch.  Global Z2/r0 via a small fp32 reduce-matmul; variance
    recombined per chunk in fp32 on [32,512] tiles (cancellation error
    ~1e-3 absolute -> STD_FLOOR 2e-3).
  - 1/Z2 via DVE reciprocal on a DMA-transposed [128,4] layout (Z2
    reaches ~1e26: ACT Ln breaks above ~1e19; [1,512] DVE divide is
    8 cyc/elem).  1/Z for pass A via exp(-ln Z) on ACT (Z <= 4096).
Step structure is software-pipelined: pass B2 of step s-1 is emitted
between pass A and pass B1 of step s so PE has independent work while
the per-item epilogue chains resolve.
"""
import sys
import numpy as np

sys.path.insert(0, "/opt/trn_rl_repo")

QB = 512          # query block = matmul free dim = one PSUM bank of fp32
KG = 256          # key group (2 chunks of 128 partitions)
NCORES = 8
PENALTY = 0.01
BIGM = 60000.0    # fp16-representable mask value
STD_FLOOR = 2.0e-3

INPUT_NAMES = ("kA", "vAr", "qA", "kB", "kmom", "cst", "qidx", "qmask")


def _ceil(a, b):
    return -(-a // b)


class _Item:
    __slots__ = ("qi", "ki", "Lq", "Lk", "qb", "ga", "gb", "dummy")

    def __init__(self, qi, ki, Lq, Lk, qb):
        self.qi, self.ki, self.Lq, self.Lk, self.qb = qi, ki, Lq, Lk, qb
        self.ga = _ceil(Lk, KG)
        self.gb = _ceil(Lq, KG)
        self.dummy = False


class _Dummy:
    qi = ki = Lq = Lk = qb = 0
    ga = gb = 0
    dummy = True


def _pack_steps(items, ns):
    """Greedy 2-D packing + local search: assign items to ns steps x 8
    cores minimizing sum_s max_core(ga) + 1.6 * sum_s max_core(gb)."""
    CB = 1.6
    steps = [[] for _ in range(ns)]
    for it in sorted(items, key=lambda t: -(t.ga + CB * t.gb)):
        best, bcost = None, None
        for s in range(ns):
            if len(steps[s]) >= NCORES:
                continue
            ga_m = max((x.ga for x in steps[s]), default=0)
            gb_m = max((x.gb for x in steps[s]), default=0)
            cost = (max(ga_m, it.ga) - ga_m + CB * (max(gb_m, it.gb) - gb_m))
            key = (cost, -len(steps[s]))
            if bcost is None or key < bcost:
                bcost, best = key, s
        steps[best].append(it)

    def _cost():
        return (sum(max((x.ga for x in s), default=0) for s in steps)
                + CB * sum(max((x.gb for x in s), default=0) for s in steps))

    base = _cost()
    for _ in range(200):
        improved = False
        for si in range(ns):
            for sj in range(ns):
                if si == sj:
                    continue
                for i in range(len(steps[si])):
                    if len(steps[sj]) < NCORES:
                        it = steps[si].pop(i)
                        steps[sj].append(it)
                        c = _cost()
                        if c < base - 1e-9:
                            base = c
                            improved = True
                            break
                        steps[sj].pop()
                        steps[si].insert(i, it)
                    done = False
                    for j in range(len(steps[sj])):
                        steps[si][i], steps[sj][j] = steps[sj][j], steps[si][i]
                        c = _cost()
                        if c < base - 1e-9:
                            base = c
                            improved = done = True
                            break
                        steps[si][i], steps[sj][j] = steps[sj][j], steps[si][i]
                    if done:
                        break
                else:
                    continue
                break
        if not improved:
            break
    for s in range(ns):
        while len(steps[s]) < NCORES:
            steps[s].append(_Dummy())
    return steps


def _split_hi_lo16(v):
    hi = v.astype(np.float16).astype(np.float32)
    lo = (v - hi).astype(np.float16).astype(np.float32)
    return hi, lo


def _bf16r(a):
    b = np.asarray(a, np.float32).copy()
    v = b.view(np.uint32)
    v += 0x8000
    v &= 0xFFFF0000
    return b


def pack(seq, src_len, combinations):
    """Build the step plan and per-core input arrays.

    Per-core inputs:
      kA   fp16 [36, CA]    pass-A keys [y; y2h; y2l; 1; 1] (masked y2h=BIGM)
      vAr  fp16 [128, CA//128*33]  pass-A values pre-swizzled:
                            vAr[p, c*33+d] = vA[c*128+p, d], vA = [y | 1]
      qA   fp16 [36, QB*NS] pass-A queries [2xT; -1; -1; -x2h; -x2l]
      kB   fp16 [34, CB]    pass-B keys [2xT; x2h; x2l] (masked x2h=BIGM)
      kmom bf16 [128, 32*128] chunk-moment lhsT: for chunk ch, col ch =
                            1, col 32+ch = -2p, col 64+ch = 128*(p*p>>7),
                            col 96+ch = p*p&127; zeros elsewhere
      cst  fp32 [64, 35]    col 0-32: redW (Z2/r0 reduce lhsT),
                            col 33 rows 0-31: -128*ch, col 34 rows 0-31: 1
      qidx/qmask fp32 [128, 4*NS] absolute query index / valid mask
    """
    seq = np.asarray(seq, np.float32)
    lens = (np.asarray(src_len).astype(np.int64) // 4).astype(np.int64)
    comb = np.asarray(combinations).astype(np.int64)

    items = []
    for a, b in comb:
        for qi, ki in ((a, b), (b, a)):
            Lq, Lk = int(lens[qi]), int(lens[ki])
            if Lq <= 0 or Lk <= 0:
                continue
            for qb in range(_ceil(Lq, QB)):
                items.append(_Item(int(qi), int(ki), Lq, Lk, qb))
    NS = max(1, _ceil(len(items), NCORES))
    steps = _pack_steps(items, NS)

    GA = [max(max(it.ga for it in steps[s]), 1) for s in range(NS)]
    GB = [max(max(it.gb for it in steps[s]), 1) for s in range(NS)]
    CA = sum(GA) * KG
    CB = sum(GB) * KG

    sq2 = np.einsum("btd,btd->bt", seq, seq).astype(np.float32)

    p = np.arange(128, dtype=np.float32)
    kmom = np.zeros((128, 32 * 128), np.float32)
    for ch in range(32):
        kmom[:, 128 * ch + ch] = 1.0
        kmom[:, 128 * ch + 32 + ch] = -2.0 * p
        kmom[:, 128 * ch + 64 + ch] = 128.0 * np.floor(p * p / 128)
        kmom[:, 128 * ch + 96 + ch] = p * p - 128.0 * np.floor(p * p / 128)
    cst = np.zeros((64, 35), np.float32)
    cst[0:32, 0] = 1.0                       # Z2 = sum Z2c
    cst[0:32, 32] = 128.0 * np.arange(32)    # r0 += 128*ch*Z2c
    cst[32:64, 32] = -0.5                    # r0 += -0.5 * (-2 sum P2 p)
    cst[0:32, 33] = -128.0 * np.arange(32)   # koff
    cst[0:32, 34] = 1.0                      # ones32 (stdsum reduce lhsT)

    cores = []
    for c in range(NCORES):
        kA = np.zeros((36, CA), np.float32)
        vA = np.zeros((CA, 33), np.float32)
        qA = np.zeros((36, QB * NS), np.float32)
        kB = np.zeros((34, CB), np.float32)
        qidx = np.zeros((128, 4 * NS), np.float32)
        qmask = np.zeros((128, 4 * NS), np.float32)
        offa = 0
        offb = 0
        its = []
        for s in range(NS):
            it = steps[s][c]
            its.append(it)
            na = GA[s] * KG
            nb = GB[s] * KG
            ka = kA[:, offa:offa + na]
            va = vA[offa:offa + na]
            kb = kB[:, offb:offb + nb]
            qa = qA[:, s * QB:(s + 1) * QB]
            if it.dummy:
                # scores 0 -> P = 1; Z = na via vA flag column
                va[:, 32] = 1.0
            else:
                y = seq[it.ki]
                x = seq[it.qi]
                Lk, Lq = it.Lk, it.Lq
                nk = min(Lk, na)
                ka[0:32, :nk] = y[:nk].T
                y2h, y2l = _split_hi_lo16(sq2[it.ki, :nk])
                ka[32, :nk] = y2h
                ka[33, :nk] = y2l
                ka[32, nk:] = BIGM
                ka[34, :nk] = 1.0
                ka[35, :nk] = 1.0
                va[:nk, 0:32] = y[:nk]
                va[:nk, 32] = 1.0
                q0 = it.qb * QB
                nq = min(Lq - q0, QB)
                qa[0:32, :nq] = 2.0 * x[q0:q0 + nq].T
                qa[32, :nq] = -1.0
                qa[33, :nq] = -1.0
                x2h, x2l = _split_hi_lo16(sq2[it.qi, q0:q0 + nq])
                qa[34, :nq] = -x2h
                qa[35, :nq] = -x2l
                nkb = min(Lq, nb)
                kb[0:32, :nkb] = 2.0 * x[:nkb].T
                xh, xl = _split_hi_lo16(sq2[it.qi, :nkb])
                kb[32, :nkb] = xh
                kb[33, :nkb] = xl
                kb[32, nkb:] = BIGM
                for c4 in range(4):
                    ii = q0 + c4 * 128 + np.arange(128)
                    qidx[:, s * 4 + c4] = ii.astype(np.float32)
                    qmask[:, s * 4 + c4] = (ii < Lq).astype(np.float32)
            offa += na
            offb += nb
        vAr = np.ascontiguousarray(
            vA.reshape(CA // 128, 128, 33).transpose(1, 0, 2).reshape(128, -1))

        import ml_dtypes
        cores.append(dict(
            kA=kA.astype(np.float16),
            vAr=vAr.astype(np.float16),
            qA=qA.astype(np.float16),
            kB=kB.astype(np.float16),
            kmom=_bf16r(kmom).astype(ml_dtypes.bfloat16),
            cst=cst,
            qidx=qidx, qmask=qmask, items=its))
    plan = dict(NS=NS, GA=GA, GB=GB, CA=CA, CB=CB)
    return plan, cores


def build_program(plan, debug=False):
    """Build the SPMD Bass program for the given step plan."""
    import concourse.bass as bass
    import concourse.bacc as bacc
    import concourse.mybir as mybir
    import concourse.tile as tile

    F32 = mybir.dt.float32
    F16 = mybir.dt.float16
    BF16 = mybir.dt.bfloat16
    AFT = mybir.ActivationFunctionType
    NS, GA, GB = plan["NS"], plan["GA"], plan["GB"]
    CA, CB = plan["CA"], plan["CB"]
    GAmax = max(GA)
    GBmax = max(GB)

    nc = bacc.Bacc("TRN2", target_bir_lowering=False, debug=False,
                   num_devices=NCORES)
    kA_d = nc.dram_tensor("kA", [36, CA], F16, kind="ExternalInput")
    vAr_d = nc.dram_tensor("vAr", [128, (CA // 128) * 33], F16,
                           kind="ExternalInput")
    qA_d = nc.dram_tensor("qA", [36, QB * NS], F16, kind="ExternalInput")
    kB_d = nc.dram_tensor("kB", [34, CB], F16, kind="ExternalInput")
    kmom_d = nc.dram_tensor("kmom", [128, 32 * 128], BF16,
                            kind="ExternalInput")
    cst_d = nc.dram_tensor("cst", [64, 35], F32, kind="ExternalInput")
    qidx_d = nc.dram_tensor("qidx", [128, 4 * NS], F32, kind="ExternalInput")
    qmask_d = nc.dram_tensor("qmask", [128, 4 * NS], F32,
                             kind="ExternalInput")
    out_d = nc.dram_tensor("out", [1, 1], F32, kind="ExternalOutput")
    if debug:
        dbgSu_d = nc.dram_tensor("dbgSu", [128, 4 * NS], F32,
                                 kind="ExternalOutput")
        dbgSs_d = nc.dram_tensor("dbgSs", [128, 4 * NS], F32,
                                 kind="ExternalOutput")

    with tile.TileContext(nc) as tc:
        with (
            tc.tile_pool(name="keys", bufs=2) as keys_pool,
            tc.tile_pool(name="vals", bufs=2) as vals_pool,
            tc.tile_pool(name="qrys", bufs=2) as qrys_pool,
            tc.tile_pool(name="pa", bufs=3) as pa_pool,
            tc.tile_pool(name="epi", bufs=2) as epi_pool,
            tc.tile_pool(name="r2", bufs=2) as r2_pool,
            tc.tile_pool(name="b2", bufs=2) as b2_pool,
            tc.tile_pool(name="fin", bufs=1) as fin_pool,
            tc.tile_pool(name="sc_ps", bufs=2, space="PSUM") as sc_psum,
            tc.tile_pool(name="na_ps", bufs=1, space="PSUM") as na_psum,
            tc.tile_pool(name="t2_ps", bufs=2, space="PSUM") as t2_psum,
            tc.tile_pool(name="red_ps", bufs=1, space="PSUM") as red_psum,
        ):
            kmom = fin_pool.tile([128, 32 * 128], BF16)
            nc.sync.dma_start(kmom[:], kmom_d[:])
            cst = fin_pool.tile([64, 35], F32)
            nc.sync.dma_start(cst[:], cst_d[:])
            qidx = fin_pool.tile([128, 4 * NS], F32)
            nc.sync.dma_start(qidx[:], qidx_d[:])
            qmask = fin_pool.tile([128, 4 * NS], F32)
            nc.sync.dma_start(qmask[:], qmask_d[:])
            stats_u = fin_pool.tile([128, 4 * NS], F32)
            stats_s = fin_pool.tile([128, 4 * NS], F32)
            ones128 = fin_pool.tile([128, 1], F32)
            nc.gpsimd.memset(ones128[:], 1.0)

            # carried from B1(s) into B2(s) emitted next iteration
            carry = None  # (T2, u0, rz2, s)

            offa = 0
            offb = 0
            for s in range(NS + 1):
                if s < NS:
                    ga, gb = GA[s], GB[s]
                    na, nb = ga * KG, gb * KG
                    # ---- load this step's operands
                    kA_t = keys_pool.tile([36, GAmax * KG], F16, tag="kA")
                    nc.sync.dma_start(kA_t[:, :na], kA_d[:, offa:offa + na])
                    vA_t = vals_pool.tile([128, GAmax * 66], F16, tag="vA")
                    nc.sync.dma_start(
                        vA_t[:, :ga * 66],
                        vAr_d[:, (offa // 128) * 33:((offa + na) // 128) * 33])
                    qA_t = qrys_pool.tile([36, QB], F16, tag="qA")
                    nc.sync.dma_start(qA_t[:], qA_d[:, s * QB:(s + 1) * QB])
                    kB_t = keys_pool.tile([34, GBmax * KG], F16, tag="kB")
                    nc.sync.dma_start(kB_t[:, :nb], kB_d[:, offb:offb + nb])

                    # ---- pass A (skew-2: snn MMs for group g-2 follow
                    # score MMs for group g so PE never waits on exp)
                    numA = na_psum.tile([33, QB], F32)
                    Ps = [None, None, None]

                    def snn_mms(g):
                        P = Ps[g % 3]
                        for h in range(2):
                            nc.tensor.matmul(
                                numA[:],
                                vA_t[:, (2 * g + h) * 33:(2 * g + h + 1) * 33],
                                P[:, h * QB:(h + 1) * QB],
                                start=(g == 0 and h == 0),
                                stop=(g == ga - 1 and h == 1))

                    for g in range(ga):
                        sc = sc_psum.tile([128, 2 * QB], F32, tag="sc")
                        for h in range(2):
                            ch = 2 * g + h
                            nc.tensor.matmul(
                                sc[:, h * QB:(h + 1) * QB],
                                kA_t[:, ch * 128:(ch + 1) * 128], qA_t[:],
                                start=True, stop=True)
                        if g >= 2:
                            snn_mms(g - 2)
                        P = pa_pool.tile([128, 2 * QB], BF16, tag="pa")
                        Ps[g % 3] = P
                        nc.scalar.activation(P[:], sc[:], AFT.Exp)
                    for g in range(max(0, ga - 2), ga):
                        snn_mms(g)

                    # ---- epilogue A: R2 = [snn/Z; -1; -1] fp16
                    lnz = epi_pool.tile([1, QB], F32, tag="lnz")
                    nc.scalar.activation(lnz[:], numA[32:33, :], AFT.Ln)
                    rz = epi_pool.tile([1, QB], F32, tag="rz")
                    nc.scalar.activation(rz[:], lnz[:], AFT.Exp, scale=-1.0)
                    rb = epi_pool.tile([32, QB], F32, tag="rb")
                    nc.gpsimd.partition_broadcast(rb[:], rz[:])
                    R2 = r2_pool.tile([34, QB], F16, tag="R2")
                    nc.gpsimd.memset(R2[32:34, :], -1.0)
                    nc.vector.tensor_mul(R2[0:32, :], numA[0:32, :], rb[:])

                # ---- pass B2 of the PREVIOUS step (pipelined):
                # std*Z2 = sum_ch E^2*Z2c + E*(-2 sum P2 p) + sum P2 p^2,
                # E[ch,i] = u_i - 128*ch
                if carry is not None:
                    pT2, pu0, prz2, ps = carry
                    rbU = b2_pool.tile([32, QB], F32, tag="rbU")
                    nc.gpsimd.partition_broadcast(rbU[:], pu0[:])
                    E = b2_pool.tile([32, QB], F32, tag="E")
                    nc.vector.tensor_scalar_add(E[:], rbU[:],
                                                cst[0:32, 33:34])
                    E2 = b2_pool.tile([32, QB], F32, tag="E2")
                    nc.vector.tensor_mul(E2[:], E[:], E[:])
                    c1 = b2_pool.tile([32, QB], F32, tag="c1")
                    nc.vector.tensor_mul(c1[:], E2[:], pT2[0:32, :])
                    c2 = b2_pool.tile([32, QB], F32, tag="c2")
                    nc.vector.tensor_mul(c2[:], E[:], pT2[32:64, :])
                    s1 = b2_pool.tile([32, QB], F32, tag="s1")
                    nc.vector.tensor_add(s1[:], c1[:], c2[:])
                    s2 = b2_pool.tile([32, QB], F32, tag="s2")
                    nc.vector.tensor_add(s2[:], s1[:], pT2[64:96, :])
                    s3 = b2_pool.tile([32, QB], F32, tag="s3")
                    nc.vector.tensor_add(s3[:], s2[:], pT2[96:128, :])
                    sdout = red_psum.tile([33, QB], F32, tag="red")
                    nc.tensor.matmul(sdout[0:1, :], cst[0:32, 34:35], s3[:],
                                     start=True, stop=True)
                    sstd = epi_pool.tile([1, QB], F32, tag="sstd")
                    nc.vector.tensor_mul(sstd[:], sdout[0:1, :], prz2[:])
                    for c4 in range(4):
                        nc.sync.dma_start(
                            stats_u[:, ps * 4 + c4:ps * 4 + c4 + 1],
                            pu0[0:1, c4 * 128:(c4 + 1) * 128])
                        nc.sync.dma_start(
                            stats_s[:, ps * 4 + c4:ps * 4 + c4 + 1],
                            sstd[0:1, c4 * 128:(c4 + 1) * 128])
                    carry = None

                if s >= NS:
                    break

                # ---- pass B1: chunk moments T2 (skew-2 like pass A)
                T2 = t2_psum.tile([128, QB], F32, tag="T2")
                P2s = [None, None, None]

                def mom_mms(g):
                    P2 = P2s[g % 3]
                    for h in range(2):
                        ch = 2 * g + h
                        nc.tensor.matmul(
                            T2[:],
                            kmom[:, ch * 128:(ch + 1) * 128],
                            P2[:, h * QB:(h + 1) * QB],
                            start=(g == 0 and h == 0),
                            stop=(g == gb - 1 and h == 1))

                for g in range(gb):
                    sc = sc_psum.tile([128, 2 * QB], F32, tag="sc")
                    for h in range(2):
                        ch = 2 * g + h
                        nc.tensor.matmul(
                            sc[:, h * QB:(h + 1) * QB],
                            kB_t[:, ch * 128:(ch + 1) * 128], R2[:],
                            start=True, stop=True)
                    if g >= 2:
                        mom_mms(g - 2)
                    P2 = pa_pool.tile([128, 2 * QB], BF16, tag="pb")
                    P2s[g % 3] = P2
                    nc.scalar.activation(P2[:], sc[:], AFT.Exp)
                for g in range(max(0, gb - 2), gb):
                    mom_mms(g)

                # ---- epilogue B1: Z2/r0 reduce, u = r0/Z2
                tt2 = epi_pool.tile([64, QB], F32, tag="tt2")
                nc.vector.tensor_copy(tt2[:], T2[0:64, :])
                R = red_psum.tile([33, QB], F32, tag="red")
                nc.tensor.matmul(R[:], cst[:, 0:33], tt2[:],
                                 start=True, stop=True)
                z2r = epi_pool.tile([1, QB], F32, tag="z2r")
                nc.vector.tensor_copy(z2r[:], R[0:1, :])
                z2t = epi_pool.tile([128, 4], F32, tag="z2t")
                for c4 in range(4):
                    nc.sync.dma_start(z2t[:, c4:c4 + 1],
                                      z2r[0:1, c4 * 128:(c4 + 1) * 128])
                rz2t = epi_pool.tile([128, 4], F32, tag="rz2t")
                nc.vector.reciprocal(rz2t[:], z2t[:])
                rz2 = epi_pool.tile([1, QB], F32, tag="rz2")
                for c4 in range(4):
                    nc.sync.dma_start(rz2[0:1, c4 * 128:(c4 + 1) * 128],
                                      rz2t[:, c4:c4 + 1])
                u0 = epi_pool.tile([1, QB], F32, tag="u0")
                nc.vector.tensor_mul(u0[:], R[32:33, :], rz2[:])

                carry = (T2, u0, rz2, s)
                offa += na
                offb += nb

            # ---- final: li = (i-u)^2/std + 0.005*ln(std), masked sum
            W = 4 * NS
            stdc = fin_pool.tile([128, W], F32)
            nc.vector.tensor_scalar_max(stdc[:], stats_s[:], STD_FLOOR)
            rstd = fin_pool.tile([128, W], F32)
            nc.vector.reciprocal(rstd[:], stdc[:])
            delta = fin_pool.tile([128, W], F32)
            nc.vector.tensor_sub(delta[:], qidx[:], stats_u[:])
            d2 = fin_pool.tile([128, W], F32)
            nc.vector.tensor_mul(d2[:], delta[:], delta[:])
            t1 = fin_pool.tile([128, W], F32)
            nc.vector.tensor_mul(t1[:], d2[:], rstd[:])
            lg = fin_pool.tile([128, W], F32)
            nc.scalar.activation(lg[:], stdc[:], AFT.Ln)
            lgs = fin_pool.tile([128, W], F32)
            nc.vector.tensor_scalar_mul(lgs[:], lg[:], 0.5 * PENALTY)
            li = fin_pool.tile([128, W], F32)
            nc.vector.tensor_add(li[:], t1[:], lgs[:])
            lim = fin_pool.tile([128, W], F32)
            nc.vector.tensor_mul(lim[:], li[:], qmask[:])
            if debug:
                nc.sync.dma_start(dbgSu_d[:], stats_u[:])
                nc.sync.dma_start(dbgSs_d[:], stats_s[:])
            rowsum = fin_pool.tile([128, 1], F32)
            nc.vector.reduce_sum(rowsum[:], lim[:],
                                 axis=mybir.AxisListType.X)
            tot = red_psum.tile([33, QB], F32, tag="red")
            nc.tensor.matmul(tot[0:1, 0:1], rowsum[:], ones128[:],
                             start=True, stop=True)
            osb = fin_pool.tile([1, 1], F32)
            nc.vector.tensor_copy(osb[:], tot[0:1, 0:1])
            nc.sync.dma_start(out_d[:], osb[:])

    nc.compile()
    return nc


def kernel(seq, src_len, combinations):
    from concourse.bass_utils import run_bass_kernel_spmd

    plan, cores = pack(seq, src_len, combinations)
    nc = build_program(plan)
    in_maps = [{k: ci[k] for k in INPUT_NAMES} for ci in cores]
    res = run_bass_kernel_spmd(nc, in_maps, list(range(NCORES)))
    tot = np.float32(0.0)
    for c in range(NCORES):
        tot += np.float32(res.results[c]["out"][0, 0])
    n_pairs = np.asarray(combinations).shape[0]
    return np.float32(tot / np.float32(n_pairs))
